# revision 2
# baseline (speedup 1.0000x reference)
"""Bidirectional Mamba (MixerModel) Trainium2 kernel, v2.

Sharding: data-parallel over batch (8 batch elements -> 8 NeuronCores), no
collectives; backward direction consumes a host-flipped input copy and the
softmax pool is order-invariant, so the output never needs unflipping.

v2 redesign vs v1 (goal: minimize instruction count + device time):
- No T-chunking: full T=2048 tiles, so no scan carries / conv tails.
- LN mean-centering and (nw, nb) are folded host-side into the in_proj
  weights/bias (W' = W diag(nw) (I - J/64), b' = W nb), leaving only the
  rstd row to compute + broadcast on device.
- The 16-state selective scan runs as 4 groups of 4 states; each group is
  ONE tensor_tensor_scan over a [128, 4*2049] flat tile, with a zeroed
  boundary column between states breaking the recurrence chain.
- B/C row-broadcasts (state row -> 128 partitions) go through a DRAM
  scratch roundtrip: one DMA stores the xproj rows, then stride-0
  partition-broadcast DMA loads materialize [128, 4, 2048] tiles. No
  per-state broadcast matmuls.
- dbx/yprod are single 3-D DVE multiplies with a stride-0 broadcast view
  of u; state reduction is a 2-level strided add tree.
- Depthwise conv = 4 shifted scalar_tensor_tensor ops (DVE), no GPSIMD.
- Scan-side tensors in bf16 (2x DVE rate; scan state stays fp32 in-HW).
- All params packed into 2 DRAM blobs (1 f32, 1 bf16) -> 3 input DMAs.
"""

import numpy as np

D_MODEL = 64
N_LAYER = 4
D_INNER = 128
D_STATE = 16
D_CONV = 4
DT_RANK = 4
EPS = 1e-5
T = 2048
B = 8
NCORES = 8
MM = 512               # psum bank free size (fp32)
G = 4                  # states per scan group
NG = D_STATE // G      # scan groups
TB = T + 1             # per-state scan span (T + boundary column)


def _legalize_sync_waits(nc, mybir, maxw=1):
    """This container's walrus only accepts one sync-wait command per
    instruction (newer bass emits several, e.g. on the kernel-tail drain).
    Split excess waits onto preceding same-engine NOPs."""
    for blk in nc.m.functions[0].blocks:
        newlist, changed = [], False
        for inst in blk.instructions:
            si = inst.sync_info
            waits = list(si.on_wait) if si and si.on_wait else []
            if len(waits) > maxw:
                k = 0
                while len(waits) > maxw:
                    chunk, waits = waits[:maxw], waits[maxw:]
                    newlist.append(mybir.InstNoOp(
                        name=f"{inst.name}-waitsplit{k}", engine=inst.engine,
                        sync_info=mybir.SyncInfo(on_wait=chunk, on_update=[])))
                    k += 1
                inst.sync_info = mybir.SyncInfo(
                    on_wait=waits, on_update=list(si.on_update or []))
                changed = True
            newlist.append(inst)
        if changed:
            blk.instructions = newlist


# ---- packed f32 blob column map (built in lockstep host/device) ---------
def _f32_cols():
    """Yield (name, d, l, ncols). d/l = -1 for shared."""
    for l in range(N_LAYER):
        yield ("in_bx", 0, l, 1)
        yield ("in_bz", 0, l, 1)
        yield ("in_bx", 1, l, 1)
        yield ("in_bz", 1, l, 1)
        for d in range(2):
            yield ("conv_w", d, l, D_CONV)
            yield ("conv_b", d, l, 1)
            yield ("dt_b", d, l, 1)
            yield ("A", d, l, D_STATE)
            yield ("Dp", d, l, 1)
    yield ("stat", -1, -1, 2)       # mean lhsT: (1/64) block cols, f32
    yield ("wl", -1, -1, 2)         # head logits lhsT (centered, per dir)
    yield ("nfwb", -1, -1, 2)       # nfw col, nfb col (stacked 2 dirs)
    yield ("ll_lhsT", -1, -1, D_MODEL)
    yield ("ll_b", -1, -1, 1)
    yield ("eps", -1, -1, 1)
    yield ("ones", -1, -1, 1)


def _bf16_cols():
    for l in range(N_LAYER):
        yield ("in_lhsT", -1, l, 2 * D_INNER)     # [128,256] both dirs stacked
        for d in range(2):
            yield ("xproj_lhsT", d, l, 68)
            yield ("dt_lhsT", d, l, D_INNER)      # rows 64:68 live
            yield ("out_lhsT", d, l, D_MODEL)
    yield ("bcast", -1, -1, D_INNER)   # [2,128] row-select, rows 0:2 live
    yield ("statb", -1, -1, 2)         # (1/64) block cols, bf16 (for sq-mm)


def _colmap(gen):
    m, off = {}, 0
    for name, d, l, n in gen():
        m[(name, d, l)] = (off, n)
        off += n
    return m, off


F32MAP, F32COLS = _colmap(_f32_cols)
BF16MAP, BF16COLS = _colmap(_bf16_cols)


def build_nc(legalize=True):
    import concourse.bass as bass
    import concourse.mybir as mybir
    import concourse.tile as tile
    from contextlib import ExitStack

    dt32 = mybir.dt.float32
    dt16 = mybir.dt.bfloat16
    Alu = mybir.AluOpType
    Act = mybir.ActivationFunctionType

    nc = bass.Bass("TRN2", target_bir_lowering=False, debug=False,
                   num_devices=NCORES)

    xin = nc.dram_tensor("xin", [2 * D_MODEL, T], dt32,
                         kind="ExternalInput").ap()
    pf32 = nc.dram_tensor("pf32", [D_INNER, F32COLS], dt32,
                          kind="ExternalInput").ap()
    pb16 = nc.dram_tensor("pb16", [D_INNER, BF16COLS], dt16,
                          kind="ExternalInput").ap()
    # per-(d,l) DRAM scratch for the B/C broadcast roundtrip
    bcsd = nc.dram_tensor("bcsd", [2 * N_LAYER, 48, T], dt16,
                          kind="Internal").ap()
    out_d = nc.dram_tensor("out", [D_MODEL, 1], dt32,
                           kind="ExternalOutput").ap()

    with tile.TileContext(nc) as tc, ExitStack() as ctx:
        const = ctx.enter_context(tc.tile_pool(name="const", bufs=1))
        big = ctx.enter_context(tc.tile_pool(name="big", bufs=2))
        one = ctx.enter_context(tc.tile_pool(name="one", bufs=1))
        scan1 = ctx.enter_context(tc.tile_pool(name="scan1", bufs=1))
        rows = ctx.enter_context(tc.tile_pool(name="rows", bufs=1))
        pp = ctx.enter_context(tc.tile_pool(name="pp", bufs=3, space="PSUM"))
        pq = ctx.enter_context(tc.tile_pool(name="pq", bufs=2, space="PSUM"))

        P32 = const.tile([D_INNER, F32COLS], dt32, tag="p32")
        nc.sync.dma_start(out=P32, in_=pf32)
        P16 = const.tile([D_INNER, BF16COLS], dt16, tag="p16")
        nc.sync.dma_start(out=P16, in_=pb16)

        def c32(name, d=-1, l=-1, prows=D_INNER):
            off, n = F32MAP[(name, d, l)]
            return P32[0:prows, off:off + n]

        def c16(name, d=-1, l=-1, prows=D_INNER):
            off, n = BF16MAP[(name, d, l)]
            return P16[0:prows, off:off + n]

        res = big.tile([D_INNER, T], dt32, tag="res")
        nc.sync.dma_start(out=res, in_=xin)

        # persistent scan buffers: dag double-buffered (da fill of group g+1
        # overlaps scan of group g), dbx single. Boundary columns between the
        # G per-state spans are zeroed once; every later op writes only the
        # [0:T] spans and the scan itself rewrites 0 at the boundaries.
        dagbuf = [scan1.tile([D_INNER, G * TB], dt16, tag=f"dag{i}",
                             name=f"dag{i}") for i in range(2)]
        dbxp = scan1.tile([D_INNER, G * TB], dt16, tag="dbx", name="dbxp")
        for t_ in dagbuf + [dbxp]:
            t3 = t_.rearrange("p (a b) -> p a b", b=TB)
            nc.vector.memset(t3[:, :, T:TB], 0.0)

        # ---------------- layers ----------------------------------------
        for l in range(N_LAYER):
            # === LN stats (both dirs at once) ===========================
            sq = one.tile([D_INNER, T], dt16, tag="sq")
            nc.scalar.activation(sq, res, Act.Square)
            m_sb = rows.tile([2, T], dt32, tag="m")
            rstd = rows.tile([2, T], dt16, tag="rstd")
            vtmp = rows.tile([2, T], dt32, tag="vtmp")
            for j in range(T // MM):
                sj = slice(j * MM, (j + 1) * MM)
                pm = pp.tile([2, MM], dt32, tag="pp2")
                nc.tensor.matmul(pm, c32("stat"), res[:, sj],
                                 start=True, stop=True)
                nc.scalar.activation(m_sb[:, sj], pm, Act.Copy)
                pv = pp.tile([2, MM], dt32, tag="pp2")
                nc.tensor.matmul(pv, c16("statb"), sq[:, sj],
                                 start=True, stop=True)
                nc.scalar.activation(vtmp[:, sj], pv, Act.Copy)
            msq = rows.tile([2, T], dt16, tag="msq")
            nc.scalar.activation(msq, m_sb, Act.Square)
            nc.vector.tensor_sub(vtmp, vtmp, msq)        # var = E[x^2]-m^2
            nc.scalar.activation(vtmp, vtmp, Act.Sqrt, bias=c32("eps", prows=2))
            with nc.allow_low_precision("rstd bf16 feeds bf16 bcast matmul"):
                nc.vector.reciprocal(rstd, vtmp)
            # hn = res * bcast(rstd)  (centering folded into in_proj W)
            hn = one.tile([D_INNER, T], dt16, tag="hn")
            for j in range(T // MM):
                sj = slice(j * MM, (j + 1) * MM)
                pb = pq.tile([D_INNER, MM], dt32, tag="ppb")
                nc.tensor.matmul(pb, c16("bcast", prows=2), rstd[:, sj],
                                 start=True, stop=True)
                nc.vector.tensor_mul(hn[:, sj], res[:, sj], pb)

            state = {}
            for d in range(2):
                # === in_proj ============================================
                hb = slice(d * D_MODEL, (d + 1) * D_MODEL)
                wio = BF16MAP[("in_lhsT", -1, l)][0]
                lx = P16[hb, wio:wio + D_INNER]
                lz = P16[hb, wio + D_INNER:wio + 2 * D_INNER]
                xpad = one.tile([D_INNER, D_CONV - 1 + T], dt16, tag="xpad")
                nc.vector.memset(xpad[:, 0:D_CONV - 1], 0.0)
                z = one.tile([D_INNER, T], dt16, tag=f"z{d}")
                for j in range(T // MM):
                    sj = slice(j * MM, (j + 1) * MM)
                    px = pp.tile([D_INNER, MM], dt32, tag="pp")
                    nc.tensor.matmul(px, lx, hn[hb, sj], start=True, stop=True)
                    nc.scalar.activation(
                        xpad[:, D_CONV - 1 + j * MM:D_CONV - 1 + (j + 1) * MM],
                        px, Act.Identity, bias=c32("in_bx", d, l))
                    pz = pp.tile([D_INNER, MM], dt32, tag="pp")
                    nc.tensor.matmul(pz, lz, hn[hb, sj], start=True, stop=True)
                    nc.scalar.activation(z[:, sj], pz, Act.Identity,
                                         bias=c32("in_bz", d, l))

                # === causal depthwise conv + silu =======================
                cw = c32("conv_w", d, l)
                xc = one.tile([D_INNER, T], dt16, tag="xc")
                nc.vector.tensor_scalar(xc, xpad[:, 0:T], cw[:, 0:1], None,
                                        op0=Alu.mult)
                for jj in range(1, D_CONV):
                    nc.vector.scalar_tensor_tensor(
                        xc, xpad[:, jj:jj + T], cw[:, jj:jj + 1], xc,
                        op0=Alu.mult, op1=Alu.add)
                xact = one.tile([D_INNER, T], dt16, tag=f"xact{d}")
                nc.scalar.activation(xact, xc, Act.Identity,
                                     bias=c32("conv_b", d, l))  # v = xc+b
                xsig = one.tile([D_INNER, T], dt16, tag="xsig")
                nc.scalar.activation(xsig, xact, Act.Sigmoid)
                nc.vector.tensor_mul(xact, xact, xsig)          # silu

                # === xproj (B 0:16, C 32:48, dt_raw 64:68) ==============
                xev = one.tile([68, T], dt16, tag="xev")
                for j in range(T // MM):
                    sj = slice(j * MM, (j + 1) * MM)
                    pd_ = pp.tile([68, MM], dt32, tag="pp")
                    nc.tensor.matmul(pd_, c16("xproj_lhsT", d, l),
                                     xact[:, sj], start=True, stop=True)
                    nc.scalar.activation(xev[0:48, sj], pd_[0:48], Act.Copy)
                    nc.scalar.activation(xev[64:68, sj], pd_[64:68], Act.Copy)
                nc.sync.dma_start(out=bcsd[d * N_LAYER + l], in_=xev[0:48, :])

                # === dt = softplus(dt_w @ dt_raw + dt_b) ================
                dts = one.tile([D_INNER, T], dt32, tag="dts")
                for j in range(T // MM):
                    sj = slice(j * MM, (j + 1) * MM)
                    pt = pp.tile([D_INNER, MM], dt32, tag="pp")
                    nc.tensor.matmul(pt, c16("dt_lhsT", d, l)[64:68, :],
                                     xev[64:68, sj], start=True, stop=True)
                    nc.scalar.activation(dts[:, sj], pt, Act.Exp,
                                         bias=c32("dt_b", d, l))
                nc.scalar.activation(dts, dts, Act.Ln, bias=c32("ones"))
                u = one.tile([D_INNER, T], dt16, tag="u")
                nc.vector.tensor_mul(u, dts, xact)

                # === selective scan: NG groups of G states ==============
                A_c = c32("A", d, l)
                ysum = one.tile([D_INNER, T], dt16, tag="ysum")
                u3 = u.unsqueeze(1).broadcast_to([D_INNER, G, T])
                for g in range(NG):
                    use = (l * 2 + d) * NG + g
                    dag = dagbuf[use % 2]
                    dbx = dbxp
                    dag3 = dag.rearrange("p (a b) -> p a b", b=TB)
                    dbx3 = dbx.rearrange("p (a b) -> p a b", b=TB)
                    for s in range(G):
                        nc.scalar.activation(
                            dag[:, s * TB:s * TB + T], dts, Act.Exp,
                            scale=A_c[:, g * G + s:g * G + s + 1])
                    Bb = scan1.tile([D_INNER, G, T], dt16, tag="Bb",
                                    name=f"Bb_{d}_{l}_{g}")
                    Cb = scan1.tile([D_INNER, G, T], dt16, tag="Cb",
                                    name=f"Cb_{d}_{l}_{g}")
                    src = bcsd[d * N_LAYER + l]
                    nc.sync.dma_start(
                        out=Bb, in_=src[g * G:(g + 1) * G]
                        .unsqueeze(0).broadcast_to([D_INNER, G, T]))
                    nc.sync.dma_start(
                        out=Cb, in_=src[32 + g * G:32 + (g + 1) * G]
                        .unsqueeze(0).broadcast_to([D_INNER, G, T]))
                    nc.vector.tensor_mul(dbx3[:, :, 0:T], u3, Bb)
                    nc.vector.tensor_tensor_scan(dbx, dag, dbx, 0.0,
                                                 op0=Alu.mult, op1=Alu.add)
                    # yprod (into dag) and state-reduction tree
                    nc.vector.tensor_mul(dag3[:, :, 0:T], dbx3[:, :, 0:T], Cb)
                    nc.vector.tensor_add(dag3[:, 0:2, 0:T], dag3[:, 0:2, 0:T],
                                         dag3[:, 2:4, 0:T])
                    if g == 0:
                        nc.vector.tensor_add(ysum, dag3[:, 0, 0:T],
                                             dag3[:, 1, 0:T])
                    else:
                        nc.vector.tensor_add(dag3[:, 0, 0:T], dag3[:, 0, 0:T],
                                             dag3[:, 1, 0:T])
                        nc.vector.tensor_add(ysum, ysum, dag3[:, 0, 0:T])

                # === y = (x*D + ysum) * z * sigmoid(z); out_proj ========
                y = one.tile([D_INNER, T], dt16, tag="y")
                nc.vector.scalar_tensor_tensor(y, xact, c32("Dp", d, l),
                                               ysum, op0=Alu.mult, op1=Alu.add)
                zs = one.tile([D_INNER, T], dt16, tag="zs")
                nc.scalar.activation(zs, z, Act.Sigmoid)
                nc.vector.tensor_mul(y, y, z)
                nc.vector.tensor_mul(y, y, zs)
                state[d] = y
            res_new = big.tile([D_INNER, T], dt32, tag="res",
                               name=f"res_{l + 1}")
            for d in range(2):
                hb = slice(d * D_MODEL, (d + 1) * D_MODEL)
                for j in range(T // MM):
                    sj = slice(j * MM, (j + 1) * MM)
                    po = pp.tile([D_MODEL, MM], dt32, tag="pp")
                    nc.tensor.matmul(po, c16("out_lhsT", d, l),
                                     state[d][:, sj], start=True, stop=True)
                    nc.vector.tensor_add(res_new[hb, sj], po, res[hb, sj])
            res = res_new

        # ---------------- head ------------------------------------------
        # stats of final residual stream (both dirs)
        sq = one.tile([D_INNER, T], dt16, tag="sq")
        nc.scalar.activation(sq, res, Act.Square)
        m_sb = rows.tile([2, T], dt32, tag="m")
        rstd = rows.tile([2, T], dt16, tag="rstd")
        vtmp = rows.tile([2, T], dt32, tag="vtmp")
        lgt = rows.tile([2, T], dt32, tag="lgt")
        for j in range(T // MM):
            sj = slice(j * MM, (j + 1) * MM)
            pm = pp.tile([2, MM], dt32, tag="pp2")
            nc.tensor.matmul(pm, c32("stat"), res[:, sj], start=True, stop=True)
            nc.scalar.activation(m_sb[:, sj], pm, Act.Copy)
            pv = pp.tile([2, MM], dt32, tag="pp2")
            nc.tensor.matmul(pv, c16("statb"), sq[:, sj], start=True, stop=True)
            nc.scalar.activation(vtmp[:, sj], pv, Act.Copy)
            pl = pp.tile([2, MM], dt32, tag="pp2")
            nc.tensor.matmul(pl, c32("wl"), res[:, sj], start=True, stop=True)
            nc.scalar.activation(lgt[:, sj], pl, Act.Copy)
        msq = rows.tile([2, T], dt16, tag="msq")
        nc.scalar.activation(msq, m_sb, Act.Square)
        nc.vector.tensor_sub(vtmp, vtmp, msq)
        nc.scalar.activation(vtmp, vtmp, Act.Sqrt, bias=c32("eps", prows=2))
        with nc.allow_low_precision("rstd bf16 in head"):
            nc.vector.reciprocal(rstd, vtmp)
        # logits = (wl . (res - m)) * rstd   (softmax shift-invariant)
        nc.vector.tensor_mul(lgt, lgt, rstd)
        sm = rows.tile([2, 8], dt32, tag="sm")
        nc.vector.reduce_max(sm[:, 0:1], lgt, axis=mybir.AxisListType.X)
        nc.vector.tensor_scalar_mul(sm[:, 1:2], sm[:, 0:1], -1.0)
        nc.scalar.activation(lgt, lgt, Act.Exp, bias=sm[:, 1:2])
        nc.vector.reduce_sum(sm[:, 2:3], lgt, axis=mybir.AxisListType.X)
        nc.vector.reciprocal(sm[:, 3:4], sm[:, 2:3])
        wrow = rows.tile([2, T], dt16, tag="wrow")
        # w = softmax(logits) * rstd  (per-t pool weight, LN scale folded)
        nc.vector.tensor_scalar(wrow, lgt, sm[:, 3:4], None, op0=Alu.mult)
        nc.vector.tensor_mul(wrow, wrow, rstd)
        # pooled1 = sum_t w_t * res_t  [128,1]; pooled0 = sum_t w_t * m_t [2,1]
        pacc = rows.tile([D_INNER, T // MM], dt32, tag="pacc")
        for j in range(T // MM):
            sj = slice(j * MM, (j + 1) * MM)
            pb = pq.tile([D_INNER, MM], dt32, tag="ppb")
            nc.tensor.matmul(pb, c16("bcast", prows=2), wrow[:, sj],
                             start=True, stop=True)
            nc.vector.scalar_tensor_tensor(
                pb, res[:, sj], 1.0, pb, op0=Alu.mult, op1=Alu.mult,
                accum_out=pacc[:, j:j + 1])
        nc.vector.tensor_add(pacc[:, 0:2], pacc[:, 0:2], pacc[:, 2:4])
        nc.vector.tensor_add(pacc[:, 0:1], pacc[:, 0:1], pacc[:, 1:2])
        wm = rows.tile([2, T], dt16, tag="msq")
        p0 = rows.tile([2, 1], dt32, tag="p0")
        nc.vector.scalar_tensor_tensor(wm, m_sb, 1.0, wrow,
                                       op0=Alu.mult, op1=Alu.mult,
                                       accum_out=p0)
        p0n = rows.tile([2, 1], dt32, tag="p0n")
        nc.vector.tensor_scalar_mul(p0n, p0, -1.0)
        pc = pq.tile([D_INNER, 1], dt32, tag="ppb")
        with nc.allow_low_precision("bcast of -pooled0"):
            p0n16 = rows.tile([2, 1], dt16, tag="p0n16")
            nc.vector.tensor_copy(p0n16, p0n)
        nc.tensor.matmul(pc, c16("bcast", prows=2), p0n16,
                         start=True, stop=True)
        pooled = rows.tile([D_INNER, 1], dt32, tag="pooled")
        nc.vector.tensor_add(pooled, pacc[:, 0:1], pc)
        # apply folded nf scale/bias: pooled = pooled*nfw + nfb
        nfo = F32MAP[("nfwb", -1, -1)][0]
        nc.vector.scalar_tensor_tensor(pooled, pooled, P32[:, nfo:nfo + 1],
                                       P32[:, nfo + 1:nfo + 2],
                                       op0=Alu.mult, op1=Alu.add)
        pf = pp.tile([D_MODEL, 1], dt32, tag="pp")
        nc.tensor.matmul(pf, c32("ll_lhsT"), pooled, start=True, stop=True)
        out_sb = rows.tile([D_MODEL, 1], dt32, tag="outsb")
        nc.scalar.activation(out_sb, pf, Act.Identity,
                             bias=c32("ll_b", prows=D_MODEL))
        nc.sync.dma_start(out=out_d, in_=out_sb)

    if legalize:
        _legalize_sync_waits(nc, mybir)
    return nc


def prep_inputs(inputs):
    """Host-side prep: fold LN into in_proj, pack params into 2 blobs."""
    import ml_dtypes
    f = np.float32
    c = np.ascontiguousarray
    x = np.asarray(inputs["x"], f)               # [8, 64, 32, 64]
    xf = x.reshape(B, D_MODEL, T)
    xb = xf[:, :, ::-1]

    C64 = np.eye(D_MODEL, dtype=f) - 1.0 / D_MODEL

    in_w = np.asarray(inputs["in_w"], f)         # [2,4,256,64]
    nw = np.asarray(inputs["nw"], f)             # [2,4,64]
    nb = np.asarray(inputs["nb"], f)             # [2,4,64]
    conv_w = np.asarray(inputs["conv_w"], f)     # [2,4,128,4]
    conv_b = np.asarray(inputs["conv_b"], f)     # [2,4,128]
    xproj_w = np.asarray(inputs["xproj_w"], f)   # [2,4,36,128]
    dt_w = np.asarray(inputs["dt_w"], f)         # [2,4,128,4]
    dt_b = np.asarray(inputs["dt_b"], f)         # [2,4,128]
    A = -np.exp(np.asarray(inputs["A_log"], f))  # [2,4,128,16]
    Dp = np.asarray(inputs["D"], f)              # [2,4,128]
    out_w = np.asarray(inputs["out_w"], f)       # [2,4,64,128]
    nf_w = np.asarray(inputs["nf_w"], f)         # [64]
    nf_b = np.asarray(inputs["nf_b"], f)         # [64]
    fp_w = np.asarray(inputs["fp_w"], f)         # [1,64]
    bp_w = np.asarray(inputs["bp_w"], f)
    ll_w = np.asarray(inputs["ll_w"], f)         # [64,128]
    ll_b = np.asarray(inputs["ll_b"], f)         # [64]

    p32 = np.zeros((D_INNER, F32COLS), f)
    p16f = np.zeros((D_INNER, BF16COLS), f)

    def put32(name, d, l, val, prows=None):
        off, n = F32MAP[(name, d, l)]
        v = np.asarray(val, f)
        if v.ndim == 1:
            v = v[:, None]
        p32[0:v.shape[0], off:off + n] = v

    def put16(name, d, l, val):
        off, n = BF16MAP[(name, d, l)]
        v = np.asarray(val, f)
        if v.ndim == 1:
            v = v[:, None]
        p16f[0:v.shape[0], off:off + n] = v

    for l in range(N_LAYER):
        # in_lhsT: rows d*64:(d+1)*64 = C64 @ diag(nw) @ in_w[d,l].T
        blk = np.zeros((D_INNER, 2 * D_INNER), f)
        for d in range(2):
            w_eff = C64 @ (nw[d, l][:, None] * in_w[d, l].T)  # [64, 256]
            blk[d * D_MODEL:(d + 1) * D_MODEL] = w_eff
            bias = in_w[d, l] @ nb[d, l]                      # [256]
            put32("in_bx", d, l, bias[:D_INNER])
            put32("in_bz", d, l, bias[D_INNER:])
            put32("conv_w", d, l, conv_w[d, l])
            put32("conv_b", d, l, conv_b[d, l])
            put32("dt_b", d, l, dt_b[d, l])
            put32("A", d, l, A[d, l])
            put32("Dp", d, l, Dp[d, l])
            dtl = np.zeros((D_INNER, D_INNER), f)
            dtl[64:68, :] = dt_w[d, l].T                      # [4,128]
            put16("dt_lhsT", d, l, dtl)
            # xproj padded: psum rows 0:16 B, 32:48 C, 64:68 dt_raw
            xp = np.zeros((D_INNER, 68), f)
            xpT = xproj_w[d, l].T                             # [128, 36]
            xp[:, 0:D_STATE] = xpT[:, DT_RANK:DT_RANK + D_STATE]
            xp[:, 32:48] = xpT[:, DT_RANK + D_STATE:]
            xp[:, 64:68] = xpT[:, 0:DT_RANK]
            put16("xproj_lhsT", d, l, xp)
            put16("out_lhsT", d, l, out_w[d, l].T)            # [128, 64]
        put16("in_lhsT", -1, l, blk)

    stat = np.zeros((D_INNER, 2), f)
    stat[0:D_MODEL, 0] = 1.0 / D_MODEL
    stat[D_MODEL:, 1] = 1.0 / D_MODEL
    put32("stat", -1, -1, stat)
    put16("statb", -1, -1, stat)
    wl = np.zeros((D_INNER, 2), f)
    wl[0:D_MODEL, 0] = C64 @ (nf_w * fp_w[0])
    wl[D_MODEL:, 1] = C64 @ (nf_w * bp_w[0])
    put32("wl", -1, -1, wl)
    nfwb = np.zeros((D_INNER, 2), f)
    nfwb[:, 0] = np.concatenate([nf_w, nf_w])
    nfwb[:, 1] = np.concatenate([nf_b, nf_b])
    put32("nfwb", -1, -1, nfwb)
    put32("ll_lhsT", -1, -1, ll_w.T)                          # [128, 64]
    put32("ll_b", -1, -1, ll_b)
    put32("eps", -1, -1, np.full((D_INNER,), EPS, f))
    put32("ones", -1, -1, np.ones((D_INNER,), f))
    bc = np.zeros((D_INNER, D_INNER), f)
    bc[0, 0:D_MODEL] = 1.0
    bc[1, D_MODEL:] = 1.0
    put16("bcast", -1, -1, bc)

    p16 = p16f.astype(ml_dtypes.bfloat16)
    in_maps = []
    for b in range(B):
        in_maps.append({
            "xin": c(np.concatenate([xf[b], xb[b]], axis=0)),
            "pf32": p32,
            "pb16": p16,
        })
    return in_maps


def kernel(**inputs):
    from concourse.bass_utils import run_bass_kernel_spmd
    in_maps = prep_inputs(inputs)
    nc = build_nc()
    res = run_bass_kernel_spmd(nc, in_maps, core_ids=list(range(NCORES)))
    out = np.stack([res.results[b]["out"][:, 0] for b in range(B)])
    return out.astype(np.float32)
